# revision 8
# baseline (speedup 1.0000x reference)
"""Dynamic depthwise 3x3 conv (per-pixel weights) on 8 TRN2 NeuronCores.

out[n,c,y,x] = sum_{ki,kj} xpad[n,c,y+ki-1,x+kj-1] * w[n, c*9+3*ki+kj, y, x]

Sharding: pure data parallel over N=8 (one image per core).

Per-core design (v3, bf16 compute): C=128 on partitions. All loads are
gpsimd (SWDGE) casting DMAs that read f32 HBM and write bf16 SBUF —
measured at the same ~280us/pass floor as plain HWDGE f32 loads, while
halving the SBUF-write bytes and unlocking DVE's 2x bf16 perf mode. The
f32 kernel was stuck at ~420us because full-f32 compute (~430 MB/pass of
SBUF engine traffic) contends with the DMA stream's SBUF writes; bf16
halves both sides (compute-side precision loss ~0.1%, well inside the
2e-2 gate).

The whole x image lives in SBUF (bf16) in a halo layout: row r occupies
slot r+1 at free-offset 1 + (r+1)*RS with RS = W+1, so the single zero
element between consecutive rows serves as both the right-pad of one row
and the left-pad of the next, and slots 0 / H+1 are persistent zero rows.
Every tap then reads a full-width shifted 3D AP with no boundary
special-casing. x is loaded once per pass as 4 chunk DMAs interleaved
into the weight-load stream.

H is processed in 16 blocks of R=8 rows. Weights load as 3-tap group
DMAs (6 KiB/partition bf16) into a 12-deep ring = 4 blocks of prefetch.
Work split per block: DVE owns taps 0-5, their partial-sum chain, and
the final combine (written f32); GpSimd owns taps 6-8 and its chain.
DVE and GpSimd get separate product/sum pools so ring reuse never
couples the two engines' schedules. Result-dependent stores issue from
ACT's HWDGE queue, so they can never head-of-line-block the loads.
"""

import numpy as np

import concourse.bass as bass
import concourse.bacc as bacc
import concourse.mybir as mybir
from concourse import tile

N, C, H, W = 8, 128, 128, 128
R = 8  # rows per block
_XS_PENDING = {}  # staged x chunks awaiting their halo copy (build-time only)
NBLOCKS = H // R
NCHUNKS = 4  # x-image load chunks per pass
CROWS = H // NCHUNKS  # rows per x chunk
RS = W + 1  # row stride inside the persistent x image (shared zero gap col)
FREE_X = 1 + (H + 2) * RS + 1  # leading zero + H+2 slots + tail slack
F32 = mybir.dt.float32
BF16 = mybir.dt.bfloat16
MULT = mybir.AluOpType.mult
ADD = mybir.AluOpType.add


def _rows3d(ap, start, nrows):
    """[128, nrows, W] view of the x image at free-offset `start`, row stride RS."""
    return ap[:, start : start + nrows * RS].rearrange("p (r c) -> p r c", c=RS)[
        :, :, 0:W
    ]


def _emit_block(nc, pools, x_f, x_d, w_d, o_d, b):
    wpool, vppool, vspool, opool, xspool = pools
    y0 = b * R

    # x chunk j covers rows 32j..32j+31; the first reader of its FIRST row
    # is block 4j+3 (its ki=2 taps), so chunk j+1's halo copy must be
    # emitted no later than block 4j+3 — a reader emitted before the write
    # would be ordered WAR in front of it and read zeros. A direct SWDGE
    # load into the RS-strided halo layout costs 4096 512B descriptors per
    # chunk (~6us of descriptor processing each); instead SWDGE cast-loads
    # the chunk CONTIGUOUSLY (1 descriptor/partition) one block early, and
    # DVE — which has ~100us/pass of slack — does the strided copy into
    # the halo layout.
    bpc = NBLOCKS // NCHUNKS
    js = 0 if b == 0 else (b // bpc) + 1 if b % bpc == bpc - 2 else None
    jc = 0 if b == 0 else (b // bpc) + 1 if b % bpc == bpc - 1 else None
    if js is not None and js < NCHUNKS:
        r0 = js * CROWS
        xs_t = xspool.tile([C, CROWS, W], BF16, tag="xs", name=f"xs_{js}")
        nc.gpsimd.dma_start(out=xs_t[:], in_=x_d[:, r0 : r0 + CROWS, :])
        _XS_PENDING[js] = xs_t
    if jc is not None and jc < NCHUNKS:
        r0 = jc * CROWS
        nc.vector.tensor_copy(
            out=_rows3d(x_f, 1 + (r0 + 1) * RS, CROWS),
            in_=_XS_PENDING.pop(jc)[:],
        )

    # one 9-tap casting load per block: f32 DRAM -> bf16 SBUF. One SWDGE
    # trigger (~1us serialized on the Pool sequencer) instead of three.
    w9 = wpool.tile([C, 9, R, W], BF16, tag="w", name=f"w_{y0}")
    nc.gpsimd.dma_start(out=w9[:], in_=w_d[:, :, y0 : y0 + R, :])
    w_ts = [w9[:, k] for k in range(9)]

    # ALL tap compute on DVE in bf16 (2-byte dtypes double DVE throughput;
    # GpSimd measured ~3.5us per bf16 tensor op here — catastrophically
    # slow — so it only runs the SWDGE load queue). Serial accumulate:
    # acc_k = acc_{k-1} + p_k; the last add writes f32.
    v = nc.vector

    def mul(k):
        ki, kj = divmod(k, 3)
        p_t = vppool.tile([C, R, W], BF16, tag="p", name=f"p_{y0}_{k}")
        v.tensor_tensor(
            out=p_t[:],
            in0=_rows3d(x_f, (y0 + ki) * RS + kj, R),
            in1=w_ts[k],
            op=MULT,
        )
        return p_t

    def add(nm, a, b_, dt=BF16, pool=None):
        s_t = (pool or vspool).tile([C, R, W], dt, tag="o" if dt is F32 else "s",
                                    name=f"{nm}_{y0}")
        v.tensor_tensor(out=s_t[:], in0=a[:], in1=b_[:], op=ADD)
        return s_t

    p0, p1 = mul(0), mul(1)
    acc = add("s1", p0, p1)
    for k in range(2, 8):
        p = mul(k)
        acc = add(f"s{k}", acc, p)
    p8 = mul(8)
    o_t = add("o", acc, p8, dt=F32, pool=opool)
    # store on ACT's HWDGE queue
    nc.scalar.dma_start(out=o_d[:, y0 : y0 + R, :], in_=o_t[:])


def build_nc(repeat=1, bufs=(4, 6, 6, 3, 2)):
    nc = bacc.Bacc("TRN2", target_bir_lowering=False, debug=False)
    x_d = nc.dram_tensor("x", [C, H, W], F32, kind="ExternalInput")
    w_d = nc.dram_tensor("w", [C, 9, H, W], F32, kind="ExternalInput")
    o_d = nc.dram_tensor("out", [C, H, W], F32, kind="ExternalOutput")
    with tile.TileContext(nc) as tc:
        with (
            tc.tile_pool(name="xp", bufs=1) as xpool,
            tc.tile_pool(name="wp", bufs=bufs[0]) as wpool,
            tc.tile_pool(name="vp", bufs=bufs[1]) as vppool,
            tc.tile_pool(name="vs", bufs=bufs[2]) as vspool,
            tc.tile_pool(name="op", bufs=bufs[3]) as opool,
            tc.tile_pool(name="xs", bufs=bufs[4]) as xspool,
        ):
            # persistent x image; zero pads/gaps survive all passes because
            # the chunk DMAs only ever write row interiors
            x_f = xpool.tile([C, FREE_X], BF16, tag="xf", name="xf")
            nc.vector.memset(x_f[:], 0.0)
            pools = (wpool, vppool, vspool, opool, xspool)

            def body():
                for b in range(NBLOCKS):
                    _emit_block(nc, pools, x_f, x_d, w_d, o_d, b)

            if repeat == 1:
                body()
            else:
                with tc.For_i(0, repeat, 1):
                    body()
    nc.compile()
    return nc


def make_runner(nc):
    """One jitted single-core executable for `nc` (no collectives, no
    partition id). Returns (fn, in_names, out_names, zero_outs); call
    `fn(*inputs, *donated_zero_outs)` with all arrays resident on ONE
    device — execution runs on that device, dispatch is async.

    This deliberately avoids run_bass_kernel_spmd's shard_map path: the
    global concat + per-device dynamic-slice it generates compiles into a
    pathologically large XLA-Neuron program. Independent per-device jits
    sidestep that entirely.
    """
    import jax

    from concourse.bass2jax import (
        _bass_exec_p,
        install_neuronx_cc_hook,
        partition_id_tensor,
    )

    install_neuronx_cc_hook()
    assert not nc.has_collectives
    part_name = nc.partition_id_tensor.name if nc.partition_id_tensor else None
    in_names, out_names, out_avals, zero_outs = [], [], [], []
    for alloc in nc.m.functions[0].allocations:
        if not isinstance(alloc, mybir.MemoryLocationSet):
            continue
        name = alloc.memorylocations[0].name
        if alloc.kind == "ExternalInput":
            if name == part_name:
                continue
            in_names.append(name)
        elif alloc.kind == "ExternalOutput":
            np_dt = mybir.dt.np(alloc.dtype)
            out_avals.append(jax.core.ShapedArray(tuple(alloc.tensor_shape), np_dt))
            out_names.append(name)
            zero_outs.append(np.zeros(tuple(alloc.tensor_shape), np_dt))
    n_params = len(in_names)
    all_in = tuple(
        in_names + out_names + ([part_name] if part_name is not None else [])
    )

    def _body(*args):
        operands = list(args)
        if part_name is not None:
            operands.append(partition_id_tensor())
        return tuple(
            _bass_exec_p.bind(
                *operands,
                out_avals=tuple(out_avals),
                in_names=all_in,
                out_names=tuple(out_names),
                lowering_input_output_aliases=(),
                sim_require_finite=True,
                sim_require_nnan=True,
                nc=nc,
            )
        )

    donate = tuple(range(n_params, n_params + len(out_names)))
    fn = jax.jit(_body, donate_argnums=donate, keep_unused=True)
    return fn, in_names, out_names, zero_outs


_CACHE = {}


def kernel(x: np.ndarray, conv_weights: np.ndarray) -> np.ndarray:
    assert x.shape == (N, C, H, W) and conv_weights.shape == (N, C * 9, H, W)
    import jax

    if "runner" not in _CACHE:
        _CACHE["runner"] = make_runner(build_nc())
    fn, in_names, out_names, zero_outs = _CACHE["runner"]
    devices = jax.devices()[:N]

    futures = []
    for i in range(N):
        per_core = {
            "x": np.ascontiguousarray(x[i], dtype=np.float32),
            "w": np.ascontiguousarray(
                conv_weights[i].reshape(C, 9, H, W), dtype=np.float32
            ),
        }
        args = [jax.device_put(per_core[nm], devices[i]) for nm in in_names]
        args += [jax.device_put(z, devices[i]) for z in zero_outs]
        futures.append(fn(*args))
    outs = [np.asarray(f[0]) for f in futures]
    return np.stack(outs).astype(np.float32)
